# revision 1
# baseline (speedup 1.0000x reference)
import sys

if "/opt/trn_rl_repo" not in sys.path:
    sys.path.insert(0, "/opt/trn_rl_repo")

from contextlib import ExitStack

import numpy as np

import concourse.mybir as mybir
from concourse import bacc
from concourse.bass_utils import run_bass_kernel_spmd
from concourse.masks import make_identity
from concourse.tile import TileContext

F32 = mybir.dt.float32
F32R = mybir.dt.float32r

B, T, C, H, D = 8, 512, 1024, 16, 64
MAX_POS = 512
TOPK = 32
P = 128
OT = C // P  # 8 channel tiles
TT = T // P  # 4 token tiles
N_CORES = 8

NEG_BIG = -1e30


def build_program():
    nc = bacc.Bacc(None, target_bir_lowering=False)

    xT_d = nc.declare_dram_parameter("xT", [P, OT, T], F32, isOutput=False)
    wq_d = nc.declare_dram_parameter("wq", [P, OT, C], F32, isOutput=False)
    wk_d = nc.declare_dram_parameter("wk", [P, OT, C], F32, isOutput=False)
    wv_d = nc.declare_dram_parameter("wv", [P, OT, C], F32, isOutput=False)
    wo_d = nc.declare_dram_parameter("wo", [P, OT, C], F32, isOutput=False)
    bqp_d = nc.declare_dram_parameter("bqp", [P, OT], F32, isOutput=False)
    bkp_d = nc.declare_dram_parameter("bkp", [P, OT], F32, isOutput=False)
    bvb_d = nc.declare_dram_parameter("bvb", [P, C], F32, isOutput=False)
    bob_d = nc.declare_dram_parameter("bob", [P, C], F32, isOutput=False)
    gates_d = nc.declare_dram_parameter("gates", [P, H], F32, isOutput=False)
    posb_d = nc.declare_dram_parameter("posb", [H, TT, P, T], F32, isOutput=False)
    out_d = nc.declare_dram_parameter("out", [T, C], F32, isOutput=True)

    Exp = mybir.ActivationFunctionType.Exp
    Copy = mybir.ActivationFunctionType.Copy
    add = mybir.AluOpType.add
    mult = mybir.AluOpType.mult

    with TileContext(nc) as tc, ExitStack() as ctx:
        const = ctx.enter_context(tc.tile_pool(name="const", bufs=1))
        wpool = ctx.enter_context(tc.tile_pool(name="wpool", bufs=2))
        xpool = ctx.enter_context(tc.tile_pool(name="xpool", bufs=1))
        proj = ctx.enter_context(tc.tile_pool(name="proj", bufs=1))
        spool4 = ctx.enter_context(tc.tile_pool(name="spool4", bufs=4))
        spool2 = ctx.enter_context(tc.tile_pool(name="spool2", bufs=2))
        spool3 = ctx.enter_context(tc.tile_pool(name="spool3", bufs=3))
        ppool = ctx.enter_context(tc.tile_pool(name="ppool", bufs=4))
        small = ctx.enter_context(tc.tile_pool(name="small", bufs=6))
        headp = ctx.enter_context(tc.tile_pool(name="headp", bufs=2))
        biasp = ctx.enter_context(tc.tile_pool(name="biasp", bufs=2))
        outp = ctx.enter_context(tc.tile_pool(name="outp", bufs=1))
        psA = ctx.enter_context(tc.tile_pool(name="psA", bufs=1, space="PSUM"))
        psS = ctx.enter_context(tc.tile_pool(name="psS", bufs=3, space="PSUM"))
        psT = ctx.enter_context(tc.tile_pool(name="psT", bufs=2, space="PSUM"))
        psO = ctx.enter_context(tc.tile_pool(name="psO", bufs=2, space="PSUM"))

        ident_f = const.tile([P, P], F32)
        make_identity(nc, ident_f)
        ident_r = const.tile([P, P], F32R)
        nc.vector.tensor_copy(ident_r[:], ident_f[:])
        gates_sb = const.tile([P, H], F32)
        nc.sync.dma_start(gates_sb[:], gates_d[:])
        bqp_sb = const.tile([P, OT], F32)
        nc.sync.dma_start(bqp_sb[:], bqp_d[:])
        bkp_sb = const.tile([P, OT], F32)
        nc.sync.dma_start(bkp_sb[:], bkp_d[:])
        bvb_sb = const.tile([P, C], F32)
        nc.sync.dma_start(bvb_sb[:], bvb_d[:])
        bob_sb = const.tile([P, C], F32)
        nc.sync.dma_start(bob_sb[:], bob_d[:])

        # ---- V projection first (f32r; no selection sensitivity) ----
        wv_sb = wpool.tile([P, OT, C], F32R, tag="w")
        nc.gpsimd.dma_start(wv_sb[:], wv_d[:].bitcast(F32R))
        xR_sb = xpool.tile([P, OT, T], F32R, tag="x")
        nc.gpsimd.dma_start(xR_sb[:], xT_d[:].bitcast(F32R))
        wq_sb = wpool.tile([P, OT, C], F32, tag="w")
        nc.sync.dma_start(wq_sb[:], wq_d[:])
        V_sb = proj.tile([P, TT, C], F32R, tag="v")
        for tt in range(TT):
            for oh in range(2):
                ps = psA.tile([P, T], F32, tag="psA")
                for kt in range(OT):
                    nc.tensor.matmul(
                        ps[:],
                        lhsT=xR_sb[:, kt, tt * P:(tt + 1) * P],
                        rhs=wv_sb[:, kt, oh * 512:(oh + 1) * 512],
                        start=(kt == 0),
                        stop=(kt == OT - 1),
                    )
                nc.vector.tensor_tensor(
                    V_sb[:, tt, oh * 512:(oh + 1) * 512], ps[:],
                    bvb_sb[:, oh * 512:(oh + 1) * 512], op=add,
                )

        # ---- Q/K projections; results split into bf16 hi/lo so the score
        # matmuls can run as 4 exact bf16 terms instead of slow fp32.
        xT_sb = xpool.tile([P, OT, T], F32, tag="x")
        nc.sync.dma_start(xT_sb[:], xT_d[:])
        wk_sb = wpool.tile([P, OT, C], F32, tag="w")
        nc.sync.dma_start(wk_sb[:], wk_d[:])
        BF16 = mybir.dt.bfloat16
        sub_op = mybir.AluOpType.subtract
        Qhi = proj.tile([P, OT, T], BF16, tag="qhi")
        Qlo = proj.tile([P, OT, T], BF16, tag="qlo")
        Khi = proj.tile([P, OT, T], BF16, tag="khi")
        Klo = proj.tile([P, OT, T], BF16, tag="klo")
        for ot in range(OT):
            for w_sb, bias_sb, hi, lo in ((wq_sb, bqp_sb, Qhi, Qlo),
                                          (wk_sb, bkp_sb, Khi, Klo)):
                ps = psA.tile([P, T], F32, tag="psA")
                for kt in range(OT):
                    nc.tensor.matmul(
                        ps[:],
                        lhsT=w_sb[:, kt, ot * P:(ot + 1) * P],
                        rhs=xT_sb[:, kt, :],
                        start=(kt == 0),
                        stop=(kt == OT - 1),
                    )
                nc.vector.tensor_scalar_add(hi[:, ot, :], ps[:], bias_sb[:, ot:ot + 1])
                nc.vector.scalar_tensor_tensor(
                    out=lo[:, ot, :], in0=ps[:], scalar=bias_sb[:, ot:ot + 1],
                    in1=hi[:, ot, :], op0=add, op1=sub_op)

        wo_sb = wpool.tile([P, OT, C], F32R, tag="w")
        nc.gpsimd.dma_start(wo_sb[:], wo_d[:].bitcast(F32R))

        # ---- attention, head pair g = (2g, 2g+1) ----
        AO_sb = proj.tile([P, OT, T], F32R, tag="ao")
        for g in range(OT):
            for hh in range(2):
                h = 2 * g + hh
                prow = 64 * hh
                # f32r matmuls reject output base-partition 64, so each head
                # accumulates into its own base-0 [64, T] bank and odd heads
                # are DMA-shifted into the upper half of AO_sb.
                ao_ps = psO.tile([64, T], F32, tag="psO")
                p_tiles = []
                sums_h = headp.tile([P, TT], F32, tag="sums")
                for it in range(TT):
                    s_ps = psS.tile([P, T], F32, tag="psS")
                    terms = ((Qhi, Khi), (Qhi, Klo), (Qlo, Khi), (Qlo, Klo))
                    for ti, (qq, kk) in enumerate(terms):
                        nc.tensor.matmul(
                            s_ps[:],
                            lhsT=qq[prow:prow + 64, g, it * P:(it + 1) * P],
                            rhs=kk[prow:prow + 64, g, :],
                            start=(ti == 0),
                            stop=(ti == 3),
                        )
                    pb_sb = biasp.tile([P, T], F32, tag="pb")
                    nc.sync.dma_start(pb_sb[:], posb_d[h, it])
                    S_sb = spool4.tile([P, T], F32, tag="S")
                    nc.vector.tensor_tensor(S_sb[:], s_ps[:], pb_sb[:], op=add)

                    # top-32 extraction: 4 rounds of max8 + match_replace.
                    # m_all collects the 32 removed values; S4 = scores with
                    # the top-32 replaced by NEG_BIG.
                    m_all = small.tile([P, 4 * 8], F32, tag="mall")
                    sc0 = spool3.tile([P, T], F32, tag="sc")
                    sc1 = spool3.tile([P, T], F32, tag="sc")
                    sc2 = spool3.tile([P, T], F32, tag="sc")
                    S4 = spool3.tile([P, T], F32, tag="sc")
                    src = S_sb
                    for r, dst4 in enumerate((sc0, sc1, sc2, S4)):
                        nc.vector.max(out=m_all[:, r * 8:(r + 1) * 8], in_=src[:])
                        nc.vector.match_replace(
                            out=dst4[:], in_to_replace=m_all[:, r * 8:(r + 1) * 8],
                            in_values=src[:], imm_value=NEG_BIG)
                        src = dst4
                    # normalizer: sum of kept weights = sum(exp(top-32 values))
                    scrapM = small.tile([P, 4 * 8], F32, tag="scrapM")
                    nc.scalar.activation(scrapM[:], m_all[:], Exp,
                                         accum_out=sums_h[:, it:it + 1])
                    # unnormalized masked weights: exp(S) - exp(S4) is nonzero
                    # exactly at the top-32 positions (bitwise cancellation).
                    E = spool2.tile([P, T], F32, tag="E")
                    nc.scalar.activation(E[:], S_sb[:], Exp)
                    E4 = spool2.tile([P, T], F32, tag="E4")
                    nc.scalar.activation(E4[:], S4[:], Exp)
                    p_u = ppool.tile([P, T], F32, tag="P")
                    nc.gpsimd.tensor_sub(p_u[:], E[:], E4[:])
                    p_tiles.append(p_u)

                # per-head batched normalizer scale = gate / sum
                inv4 = headp.tile([P, TT], F32, tag="inv4")
                nc.vector.reciprocal(inv4[:], sums_h[:])
                scl4 = headp.tile([P, TT], F32, tag="scl4")
                nc.vector.tensor_scalar(scl4[:], inv4[:], gates_sb[:, h:h + 1],
                                        None, op0=mult)
                p_r = []
                for it in range(TT):
                    pr = ppool.tile([P, T], F32R, tag="Pr")
                    nc.scalar.activation(pr[:], p_tiles[it][:], Copy,
                                         scale=scl4[:, it:it + 1])
                    p_r.append(pr)

                # transpose P and accumulate attn_out^T
                for jt in range(TT):
                    pt_ps = psT.tile([P, T], F32R, tag="psT")
                    for it in range(TT):
                        nc.tensor.transpose(
                            pt_ps[:, it * P:(it + 1) * P],
                            p_r[it][:, jt * P:(jt + 1) * P],
                            ident_r[:],
                        )
                    PT_sb = spool2.tile([P, T], F32R, tag="PT")
                    nc.scalar.copy(PT_sb[:], pt_ps[:])
                    nc.tensor.matmul(
                        ao_ps[:],
                        lhsT=V_sb[:, jt, h * 64:(h + 1) * 64],
                        rhs=PT_sb[:],
                        start=(jt == 0),
                        stop=(jt == TT - 1),
                    )
                if hh == 0:
                    nc.scalar.copy(AO_sb[0:64, g, :], ao_ps[:])
                else:
                    stage = spool2.tile([64, T], F32R, tag="stg")
                    nc.scalar.copy(stage[:], ao_ps[:])
                    nc.sync.dma_start(AO_sb[64:128, g, :], stage[:])

        # ---- output projection (f32r) ----
        for tt in range(TT):
            for oh in range(2):
                ps = psA.tile([P, T], F32, tag="psA")
                for ct in range(OT):
                    nc.tensor.matmul(
                        ps[:],
                        lhsT=AO_sb[:, ct, tt * P:(tt + 1) * P],
                        rhs=wo_sb[:, ct, oh * 512:(oh + 1) * 512],
                        start=(ct == 0),
                        stop=(ct == OT - 1),
                    )
                o_sb = outp.tile([P, T], F32, tag="o")
                nc.vector.tensor_tensor(o_sb[:], ps[:], bob_sb[:, oh * 512:(oh + 1) * 512],
                                        op=add)
                nc.sync.dma_start(out_d[tt * P:(tt + 1) * P, oh * 512:(oh + 1) * 512], o_sb[:])

    nc.compile()
    if not nc.is_finalized():
        nc.finalize()
    return nc


def prep_inputs(x, Wq, bq, Wk, bk, Wv, bv, Wo, bo, head_gates, rel_bias):
    """Host-side reshapes/transposes into the layouts the device program wants."""
    x = np.asarray(x, np.float32)
    scale = np.float32(1.0 / np.sqrt(D))

    def to_kpart(w):
        # [C_in, C_out] -> [P, OT, C_out] with c_in = kt*P + p
        return np.ascontiguousarray(
            np.asarray(w, np.float32).reshape(OT, P, C).transpose(1, 0, 2))

    wq_r = to_kpart(np.asarray(Wq, np.float32).T * scale)
    wk_r = to_kpart(np.asarray(Wk, np.float32).T)
    wv_r = to_kpart(np.asarray(Wv, np.float32).T)
    wo_r = to_kpart(np.asarray(Wo, np.float32).T)

    bqp = np.ascontiguousarray((np.asarray(bq, np.float32) * scale).reshape(OT, P).T)
    bkp = np.ascontiguousarray(np.asarray(bk, np.float32).reshape(OT, P).T)
    bvb = np.ascontiguousarray(np.tile(np.asarray(bv, np.float32)[None, :], (P, 1)))
    bob = np.ascontiguousarray(np.tile(np.asarray(bo, np.float32)[None, :], (P, 1)))
    gates = np.ascontiguousarray(
        np.tile(np.asarray(head_gates, np.float32)[None, :], (P, 1)))

    idx = np.arange(T)
    rel = idx[None, :] - idx[:, None] + (MAX_POS - 1)          # [T, T]
    pb = np.asarray(rel_bias, np.float32)[rel]                 # [T, T, H]
    posb = np.ascontiguousarray(
        pb.transpose(2, 0, 1).reshape(H, TT, P, T))            # [H, TT, P, T]

    shared = dict(wq=wq_r, wk=wk_r, wv=wv_r, wo=wo_r, bqp=bqp, bkp=bkp,
                  bvb=bvb, bob=bob, gates=gates, posb=posb)

    in_maps = []
    for b in range(B):
        xT = np.ascontiguousarray(
            x[b].T.reshape(OT, P, T).transpose(1, 0, 2))       # [P, OT, T]
        in_maps.append(dict(xT=xT, **shared))
    return in_maps


_NC_CACHE = {}


def get_program():
    if "nc" not in _NC_CACHE:
        _NC_CACHE["nc"] = build_program()
    return _NC_CACHE["nc"]


def kernel(x, Wq, bq, Wk, bk, Wv, bv, Wo, bo, head_gates, rel_bias):
    nc = get_program()
    in_maps = prep_inputs(x, Wq, bq, Wk, bk, Wv, bv, Wo, bo, head_gates, rel_bias)
    res = run_bass_kernel_spmd(nc, in_maps, list(range(N_CORES)))
    return np.stack([res.results[b]["out"] for b in range(B)], axis=0)



# revision 15
# speedup vs baseline: 1.1836x; 1.1836x over previous
import sys

if "/opt/trn_rl_repo" not in sys.path:
    sys.path.insert(0, "/opt/trn_rl_repo")

from contextlib import ExitStack

import numpy as np

import concourse.mybir as mybir
from concourse import bacc
from concourse.bass_utils import run_bass_kernel_spmd
from concourse.masks import make_identity
from concourse.tile import TileContext

F32 = mybir.dt.float32
F32R = mybir.dt.float32r
BF16 = mybir.dt.bfloat16

B, T, C, H, D = 8, 512, 1024, 16, 64
MAX_POS = 512
TOPK = 32
P = 128
OT = C // P  # 8 channel tiles
TT = T // P  # 4 token tiles
N_CORES = 8

SHIFT = 0.0  # no score shift: removals use -1e30, which works for any sign


def build_program():
    nc = bacc.Bacc(None, target_bir_lowering=False)

    xT_d = nc.declare_dram_parameter("xT", [P, OT, T], F32, isOutput=False)
    # wq/wk are pre-chunked host-side by output tile: [ot_out, P, kt, P]
    wq_d = nc.declare_dram_parameter("wq", [OT, P, OT, P], F32, isOutput=False)
    wk_d = nc.declare_dram_parameter("wk", [OT, P, OT, P], F32, isOutput=False)
    wv_d = nc.declare_dram_parameter("wv", [P, OT, C], F32, isOutput=False)
    wo_d = nc.declare_dram_parameter("wo", [P, OT, C], F32, isOutput=False)
    bqp_d = nc.declare_dram_parameter("bqp", [P, OT], F32, isOutput=False)
    bkp_d = nc.declare_dram_parameter("bkp", [P, OT], F32, isOutput=False)
    bob_d = nc.declare_dram_parameter("bob", [P, C], F32, isOutput=False)
    gates_d = nc.declare_dram_parameter("gates", [P, H], F32, isOutput=False)
    posb_d = nc.declare_dram_parameter("posb", [H, TT, P, T], F32, isOutput=False)
    out_d = nc.declare_dram_parameter("out", [T, C], F32, isOutput=True)

    Exp = mybir.ActivationFunctionType.Exp
    Identity = mybir.ActivationFunctionType.Identity
    Copy = mybir.ActivationFunctionType.Copy
    add = mybir.AluOpType.add
    mult = mybir.AluOpType.mult
    sub_op = mybir.AluOpType.subtract
    is_lt = mybir.AluOpType.is_lt
    is_ge = mybir.AluOpType.is_ge

    with TileContext(nc) as tc, ExitStack() as ctx:
        const = ctx.enter_context(tc.tile_pool(name="const", bufs=1))
        wqkp = ctx.enter_context(tc.tile_pool(name="wqkp", bufs=6))
        wvop = ctx.enter_context(tc.tile_pool(name="wvop", bufs=1))
        xpool = ctx.enter_context(tc.tile_pool(name="xpool", bufs=1))
        proj = ctx.enter_context(tc.tile_pool(name="proj", bufs=1))
        scp = ctx.enter_context(tc.tile_pool(name="scp", bufs=6))
        pbpool = ctx.enter_context(tc.tile_pool(name="pbpool", bufs=3))
        qfpool = ctx.enter_context(tc.tile_pool(name="qfpool", bufs=2))
        empool = ctx.enter_context(tc.tile_pool(name="empool", bufs=3))
        sgpool = ctx.enter_context(tc.tile_pool(name="sgpool", bufs=3))
        epool = ctx.enter_context(tc.tile_pool(name="epool", bufs=3))
        pupool = ctx.enter_context(tc.tile_pool(name="pupool", bufs=17))
        prpool = ctx.enter_context(tc.tile_pool(name="prpool", bufs=9))
        ptpool = ctx.enter_context(tc.tile_pool(name="ptpool", bufs=2))
        small = ctx.enter_context(tc.tile_pool(name="small", bufs=10))
        headp = ctx.enter_context(tc.tile_pool(name="headp", bufs=8))
        outp = ctx.enter_context(tc.tile_pool(name="outp", bufs=2))
        # PSUM (8 banks): psAT shared between projections/transposes/out-proj;
        # psS holds score tiles (pos-bias DMA preload + matmul accumulate);
        # psO per-pair attention-output accumulator.
        psAT = ctx.enter_context(tc.tile_pool(name="psAT", bufs=2, space="PSUM"))
        psS = ctx.enter_context(tc.tile_pool(name="psS", bufs=5, space="PSUM"))
        psO = ctx.enter_context(tc.tile_pool(name="psO", bufs=1, space="PSUM"))

        ident_f = const.tile([P, P], F32)
        make_identity(nc, ident_f)
        ident_b = const.tile([P, P], BF16)
        nc.vector.tensor_copy(ident_b[:], ident_f[:])
        ident_r = const.tile([P, P], F32R)
        nc.vector.tensor_copy(ident_r[:], ident_f[:])
        gates_sb = const.tile([P, H], F32)
        nc.sync.dma_start(gates_sb[:], gates_d[:])
        bqp_sb = const.tile([P, OT], F32)
        nc.sync.dma_start(bqp_sb[:], bqp_d[:])
        bkp_sb = const.tile([P, OT], F32)
        nc.sync.dma_start(bkp_sb[:], bkp_d[:])
        bob_r = const.tile([P, C], F32R)
        nc.gpsimd.dma_start(bob_r[:], bob_d[:].bitcast(F32R))

        xR_sb = xpool.tile([P, OT, T], F32R, tag="x")
        nc.gpsimd.dma_start(xR_sb[:], xT_d[:].bitcast(F32R))

        Qhi = proj.tile([P, OT, T], BF16, tag="qhi")
        Qlo = proj.tile([P, OT, T], BF16, tag="qlo")
        Khi = proj.tile([P, OT, T], BF16, tag="khi")
        Klo = proj.tile([P, OT, T], BF16, tag="klo")
        V_sb = proj.tile([P, TT, C], BF16, tag="v")
        AO_sb = proj.tile([P, OT, T], F32R, tag="ao")

        state = {"wv": None, "wo": None}
        wqk_tiles = {}

        def load_qk_chunk(ot):
            wq_ch = wqkp.tile([P, OT, P], F32R, tag="wqk")
            nc.gpsimd.dma_start(wq_ch[:], wq_d[ot].bitcast(F32R))
            wk_ch = wqkp.tile([P, OT, P], F32R, tag="wqk")
            nc.gpsimd.dma_start(wk_ch[:], wk_d[ot].bitcast(F32R))
            wqk_tiles[ot] = (wq_ch, wk_ch)

        def emit_qk_proj(ot):
            # Q^T / K^T channel tile ot -> bf16 hi/lo splits
            wq_ch, wk_ch = wqk_tiles.pop(ot)
            for w_ch, bias_sb, hi, lo in ((wq_ch, bqp_sb, Qhi, Qlo),
                                          (wk_ch, bkp_sb, Khi, Klo)):
                ps = psAT.tile([P, T], F32, tag="psAT")
                for kt in range(OT):
                    nc.tensor.matmul(
                        ps[:],
                        lhsT=w_ch[:, kt, :],
                        rhs=xR_sb[:, kt, :],
                        start=(kt == 0),
                        stop=(kt == OT - 1),
                    )
                nc.scalar.activation(hi[:, ot, :], ps[:], Identity,
                                     bias=bias_sb[:, ot:ot + 1])
                qf = qfpool.tile([P, T], F32, tag="qf")
                nc.scalar.activation(qf[:], ps[:], Identity,
                                     bias=bias_sb[:, ot:ot + 1])
                nc.gpsimd.tensor_tensor(
                    lo[:, ot, :], qf[:], hi[:, ot, :], op=sub_op)

        def emit_v_group(tt, oh):
            # V[tok, ch] tile (V bias folded into output bias host-side)
            ps = psAT.tile([P, T], F32, tag="psAT")
            for kt in range(OT):
                nc.tensor.matmul(
                    ps[:],
                    lhsT=xR_sb[:, kt, tt * P:(tt + 1) * P],
                    rhs=state["wv"][:, kt, oh * 512:(oh + 1) * 512],
                    start=(kt == 0),
                    stop=(kt == OT - 1),
                )
            nc.scalar.copy(V_sb[:, tt, oh * 512:(oh + 1) * 512], ps[:])

        # per-pair stashes for the software-pipelined tail
        pair_pu = {}     # g -> [p_u bf16 tiles x 8] (hh-major)
        pair_sums = {}   # g -> sums tile [P, 2*TT]

        def emit_pair_scores(g):
            sums_g = headp.tile([P, 2 * TT], F32, tag="sums")
            pus = []
            for hh in range(2):
                h = 2 * g + hh
                prow = 64 * hh
                for it in range(TT):
                    pb_sb = pbpool.tile([P, T], F32R, tag="pb")
                    nc.sync.dma_start(pb_sb[:], posb_d[h, it].bitcast(F32R))
                    ps = psS.tile([P, T], F32, tag="psS")
                    # positional bias (+SHIFT) staged into PSUM by an identity
                    # matmul; score matmuls accumulate on top of it
                    nc.tensor.matmul(ps[:], lhsT=ident_r[:], rhs=pb_sb[:],
                                     start=True, stop=False)
                    terms = ((Qhi, Khi), (Qhi, Klo), (Qlo, Khi))
                    for ti, (qq, kk) in enumerate(terms):
                        nc.tensor.matmul(
                            ps[:],
                            lhsT=qq[prow:prow + 64, g, it * P:(it + 1) * P],
                            rhs=kk[prow:prow + 64, g, :],
                            start=False,
                            stop=(ti == len(terms) - 1),
                        )
                    # unmasked exp (ACT, runs in parallel with selection)
                    E = epool.tile([P, T], F32, tag="E")
                    nc.scalar.activation(E[:], ps[:], Exp)
                    # exact top-32: 4x max8; removals: round 1 via DVE
                    # match_replace->0, rounds 2/3 via GpSimd threshold-zeroing
                    m_all = small.tile([P, 32], F32, tag="mall")
                    sc0 = scp.tile([P, T], F32, tag="sc")
                    sc1 = scp.tile([P, T], F32, tag="sc")
                    sc2 = scp.tile([P, T], F32, tag="sc")
                    nc.vector.max(out=m_all[:, 0:8], in_=ps[:])
                    nc.vector.match_replace(
                        out=sc0[:], in_to_replace=m_all[:, 0:8],
                        in_values=ps[:], imm_value=-1e30)
                    nc.vector.max(out=m_all[:, 8:16], in_=sc0[:])
                    nc.vector.match_replace(
                        out=sc1[:], in_to_replace=m_all[:, 8:16],
                        in_values=sc0[:], imm_value=-1e30)
                    nc.vector.max(out=m_all[:, 16:24], in_=sc1[:])
                    nc.vector.match_replace(
                        out=sc2[:], in_to_replace=m_all[:, 16:24],
                        in_values=sc1[:], imm_value=-1e30)
                    nc.vector.max(out=m_all[:, 24:32], in_=sc2[:])
                    # exp of the 32 kept values; row-sum -> normalizer
                    scrapM = small.tile([P, 32], F32, tag="scrapM")
                    nc.scalar.activation(
                        scrapM[:], m_all[:], Exp,
                        accum_out=sums_g[:, hh * TT + it:hh * TT + it + 1])
                    # masked unnormalized weights via a Sign trick (Pool has
                    # no comparison ops): sgn = sign(v32 - (1+eps)*S) is -1 on
                    # the kept set, +1 on the dropped set; E - E*sgn = 2E on
                    # kept, exactly 0 on dropped. The eps*S margin (~1e-4)
                    # keeps the 32nd element strictly on the kept side; the
                    # factor 2 is folded into the normalization scale.
                    sgn = sgpool.tile([P, T], BF16, tag="sgn")
                    nc.scalar.activation(sgn[:], ps[:],
                                         mybir.ActivationFunctionType.Sign,
                                         bias=m_all[:, 31:32],
                                         scale=-(1.0 + 5e-6))
                    Em = empool.tile([P, T], F32, tag="Em")
                    nc.gpsimd.tensor_tensor(Em[:], E[:], sgn[:], op=mult)
                    p_u = pupool.tile([P, T], BF16, tag="P")
                    nc.gpsimd.tensor_tensor(p_u[:], E[:], Em[:], op=sub_op)
                    pus.append(p_u)
            pair_pu[g] = pus
            pair_sums[g] = sums_g

        def emit_pair_tail(g):
            sums_g = pair_sums.pop(g)
            pus = pair_pu.pop(g)
            inv = headp.tile([P, 2 * TT], F32, tag="inv")
            nc.vector.reciprocal(inv[:], sums_g[:])
            ao_ps = psO.tile([P, T], F32, tag="psO")
            for hh in range(2):
                h = 2 * g + hh
                scl = headp.tile([P, TT], F32, tag="scl")
                nc.vector.tensor_scalar(scl[:], inv[:, hh * TT:(hh + 1) * TT],
                                        gates_sb[:, h:h + 1], 0.5,
                                        op0=mult, op1=mult)
                p_r = []
                for it in range(TT):
                    pr = prpool.tile([P, T], BF16, tag="Pr")
                    nc.scalar.activation(pr[:], pus[hh * TT + it][:], Copy,
                                         scale=scl[:, it:it + 1])
                    p_r.append(pr)
                for jt in range(TT):
                    pt_ps = psAT.tile([P, T], BF16, tag="psAT")
                    for it in range(TT):
                        nc.tensor.transpose(
                            pt_ps[:, it * P:(it + 1) * P],
                            p_r[it][:, jt * P:(jt + 1) * P],
                            ident_b[:],
                        )
                    PT_sb = ptpool.tile([P, T], BF16, tag="PT")
                    nc.scalar.copy(PT_sb[:], pt_ps[:])
                    nc.tensor.matmul(
                        ao_ps[64 * hh:64 * hh + 64, :],
                        lhsT=V_sb[:, jt, h * 64:(h + 1) * 64],
                        rhs=PT_sb[:],
                        start=(jt == 0),
                        stop=(jt == TT - 1),
                    )
            nc.scalar.copy(AO_sb[:, g, :], ao_ps[:])

        # ---- software-pipelined main loop ----
        load_qk_chunk(0)
        load_qk_chunk(1)
        state["wv"] = wvop.tile([P, OT, C], F32R, tag="wvo", name="wv_sb")
        nc.gpsimd.dma_start(state["wv"][:], wv_d[:].bitcast(F32R))
        for g in range(OT):
            emit_qk_proj(g)
            if g + 2 < OT:
                load_qk_chunk(g + 2)
            if g == 1:
                for tt in range(TT):
                    emit_v_group(tt, 0)
            if g == 3:
                for tt in range(TT):
                    emit_v_group(tt, 1)
            emit_pair_scores(g)
            if g == 4:
                state["wo"] = wvop.tile([P, OT, C], F32R, tag="wvo", name="wo_sb")
                nc.gpsimd.dma_start(state["wo"][:], wo_d[:].bitcast(F32R))
            if g >= 1:
                emit_pair_tail(g - 1)
        emit_pair_tail(OT - 1)

        # ---- output projection (f32r) ----
        for tt in range(TT):
            for oh in range(2):
                ps = psAT.tile([P, T], F32, tag="psAT")
                nc.tensor.matmul(ps[:], lhsT=ident_r[:],
                                 rhs=bob_r[:, oh * 512:(oh + 1) * 512],
                                 start=True, stop=False)
                for ct in range(OT):
                    nc.tensor.matmul(
                        ps[:],
                        lhsT=AO_sb[:, ct, tt * P:(tt + 1) * P],
                        rhs=state["wo"][:, ct, oh * 512:(oh + 1) * 512],
                        start=False,
                        stop=(ct == OT - 1),
                    )
                o_sb = outp.tile([P, T], F32, tag="o")
                nc.scalar.copy(o_sb[:], ps[:])
                nc.sync.dma_start(out_d[tt * P:(tt + 1) * P,
                                        oh * 512:(oh + 1) * 512], o_sb[:])

    nc.compile()
    if not nc.is_finalized():
        nc.finalize()
    return nc


def prep_inputs(x, Wq, bq, Wk, bk, Wv, bv, Wo, bo, head_gates, rel_bias):
    """Host-side reshapes/transposes into the layouts the device program wants."""
    x = np.asarray(x, np.float32)
    scale = np.float32(1.0 / np.sqrt(D))

    def to_kpart(w):
        # [C_in, C_out] -> [P, OT, C_out] with c_in = kt*P + p
        return np.ascontiguousarray(
            np.asarray(w, np.float32).reshape(OT, P, C).transpose(1, 0, 2))

    def to_kpart_chunked(w):
        # [C_in, C_out] -> [OT_out, P, OT_kt, P]
        return np.ascontiguousarray(
            np.asarray(w, np.float32).reshape(OT, P, OT, P)
            .transpose(2, 1, 0, 3))

    wq_r = to_kpart_chunked(np.asarray(Wq, np.float32).T * scale)
    wk_r = to_kpart_chunked(np.asarray(Wk, np.float32).T)
    wv_r = to_kpart(np.asarray(Wv, np.float32).T)
    wo_r = to_kpart(np.asarray(Wo, np.float32).T)

    bqp = np.ascontiguousarray((np.asarray(bq, np.float32) * scale).reshape(OT, P).T)
    bkp = np.ascontiguousarray(np.asarray(bk, np.float32).reshape(OT, P).T)
    # V bias folded into the output bias: the normalized gated weights of each
    # head sum to exactly gate_h, so attn_out carries a constant gate_h * bv_h
    # per head, which maps through Wo^T into a constant output bias.
    g64 = np.repeat(np.asarray(head_gates, np.float64), D)
    bo_eff = (np.asarray(bo, np.float64)
              + (g64 * np.asarray(bv, np.float64)) @ np.asarray(Wo, np.float64).T)
    bob = np.ascontiguousarray(
        np.tile(bo_eff.astype(np.float32)[None, :], (P, 1)))
    gates = np.ascontiguousarray(
        np.tile(np.asarray(head_gates, np.float32)[None, :], (P, 1)))

    idx = np.arange(T)
    rel = idx[None, :] - idx[:, None] + (MAX_POS - 1)          # [T, T]
    pb = np.asarray(rel_bias, np.float32)[rel] + np.float32(SHIFT)  # [T, T, H]
    posb = np.ascontiguousarray(
        pb.transpose(2, 0, 1).reshape(H, TT, P, T))            # [H, TT, P, T]

    shared = dict(wq=wq_r, wk=wk_r, wv=wv_r, wo=wo_r, bqp=bqp, bkp=bkp,
                  bob=bob, gates=gates, posb=posb)

    in_maps = []
    for b in range(B):
        xT = np.ascontiguousarray(
            x[b].T.reshape(OT, P, T).transpose(1, 0, 2))       # [P, OT, T]
        in_maps.append(dict(xT=xT, **shared))
    return in_maps


_NC_CACHE = {}


def get_program():
    if "nc" not in _NC_CACHE:
        _NC_CACHE["nc"] = build_program()
    return _NC_CACHE["nc"]


def kernel(x, Wq, bq, Wk, bk, Wv, bv, Wo, bo, head_gates, rel_bias):
    nc = get_program()
    in_maps = prep_inputs(x, Wq, bq, Wk, bk, Wv, bv, Wo, bo, head_gates, rel_bias)
    res = run_bass_kernel_spmd(nc, in_maps, list(range(N_CORES)))
    return np.stack([res.results[b]["out"] for b in range(B)], axis=0)


# revision 16
# speedup vs baseline: 1.1938x; 1.0086x over previous
import sys

if "/opt/trn_rl_repo" not in sys.path:
    sys.path.insert(0, "/opt/trn_rl_repo")

from contextlib import ExitStack

import numpy as np

import concourse.mybir as mybir
from concourse import bacc
from concourse.bass_utils import run_bass_kernel_spmd
from concourse.masks import make_identity
from concourse.tile import TileContext

F32 = mybir.dt.float32
F32R = mybir.dt.float32r
BF16 = mybir.dt.bfloat16

B, T, C, H, D = 8, 512, 1024, 16, 64
MAX_POS = 512
TOPK = 32
P = 128
OT = C // P  # 8 channel tiles
TT = T // P  # 4 token tiles
N_CORES = 8

SHIFT = 0.0  # no score shift: removals use -1e30, which works for any sign


def build_program():
    nc = bacc.Bacc(None, target_bir_lowering=False)

    xT_d = nc.declare_dram_parameter("xT", [P, OT, T], F32, isOutput=False)
    # wq/wk are pre-chunked host-side by output tile: [ot_out, P, kt, P]
    wq_d = nc.declare_dram_parameter("wq", [OT, P, OT, P], F32, isOutput=False)
    wk_d = nc.declare_dram_parameter("wk", [OT, P, OT, P], F32, isOutput=False)
    wv_d = nc.declare_dram_parameter("wv", [P, OT, C], F32, isOutput=False)
    wo_d = nc.declare_dram_parameter("wo", [P, OT, C], F32, isOutput=False)
    bqp_d = nc.declare_dram_parameter("bqp", [P, OT], F32, isOutput=False)
    bkp_d = nc.declare_dram_parameter("bkp", [P, OT], F32, isOutput=False)
    bob_d = nc.declare_dram_parameter("bob", [P, C], F32, isOutput=False)
    gates_d = nc.declare_dram_parameter("gates", [P, H], F32, isOutput=False)
    posb_d = nc.declare_dram_parameter("posb", [H, TT, P, T], F32, isOutput=False)
    out_d = nc.declare_dram_parameter("out", [T, C], F32, isOutput=True)

    Exp = mybir.ActivationFunctionType.Exp
    Identity = mybir.ActivationFunctionType.Identity
    Copy = mybir.ActivationFunctionType.Copy
    add = mybir.AluOpType.add
    mult = mybir.AluOpType.mult
    sub_op = mybir.AluOpType.subtract
    is_lt = mybir.AluOpType.is_lt
    is_ge = mybir.AluOpType.is_ge

    with TileContext(nc) as tc, ExitStack() as ctx:
        const = ctx.enter_context(tc.tile_pool(name="const", bufs=1))
        wqkp = ctx.enter_context(tc.tile_pool(name="wqkp", bufs=6))
        wvop = ctx.enter_context(tc.tile_pool(name="wvop", bufs=1))
        xpool = ctx.enter_context(tc.tile_pool(name="xpool", bufs=1))
        proj = ctx.enter_context(tc.tile_pool(name="proj", bufs=1))
        scp = ctx.enter_context(tc.tile_pool(name="scp", bufs=6))
        pbpool = ctx.enter_context(tc.tile_pool(name="pbpool", bufs=3))
        qfpool = ctx.enter_context(tc.tile_pool(name="qfpool", bufs=2))
        empool = ctx.enter_context(tc.tile_pool(name="empool", bufs=3))
        sgpool = ctx.enter_context(tc.tile_pool(name="sgpool", bufs=3))
        epool = ctx.enter_context(tc.tile_pool(name="epool", bufs=3))
        pupool = ctx.enter_context(tc.tile_pool(name="pupool", bufs=17))
        prpool = ctx.enter_context(tc.tile_pool(name="prpool", bufs=9))
        ptpool = ctx.enter_context(tc.tile_pool(name="ptpool", bufs=2))
        small = ctx.enter_context(tc.tile_pool(name="small", bufs=10))
        headp = ctx.enter_context(tc.tile_pool(name="headp", bufs=8))
        outp = ctx.enter_context(tc.tile_pool(name="outp", bufs=2))
        # PSUM (8 banks): psAT shared between projections/transposes/out-proj;
        # psS holds score tiles (pos-bias DMA preload + matmul accumulate);
        # psO per-pair attention-output accumulator.
        psAT = ctx.enter_context(tc.tile_pool(name="psAT", bufs=3, space="PSUM"))
        psS = ctx.enter_context(tc.tile_pool(name="psS", bufs=4, space="PSUM"))
        psO = ctx.enter_context(tc.tile_pool(name="psO", bufs=1, space="PSUM"))

        ident_f = const.tile([P, P], F32)
        make_identity(nc, ident_f)
        ident_b = const.tile([P, P], BF16)
        nc.vector.tensor_copy(ident_b[:], ident_f[:])
        ident_r = const.tile([P, P], F32R)
        nc.vector.tensor_copy(ident_r[:], ident_f[:])
        gates_sb = const.tile([P, H], F32)
        nc.sync.dma_start(gates_sb[:], gates_d[:])
        bqp_sb = const.tile([P, OT], F32)
        nc.sync.dma_start(bqp_sb[:], bqp_d[:])
        bkp_sb = const.tile([P, OT], F32)
        nc.sync.dma_start(bkp_sb[:], bkp_d[:])
        bob_r = const.tile([P, C], F32R)
        nc.gpsimd.dma_start(bob_r[:], bob_d[:].bitcast(F32R))

        xR_sb = xpool.tile([P, OT, T], F32R, tag="x")
        nc.gpsimd.dma_start(xR_sb[:], xT_d[:].bitcast(F32R))

        Qhi = proj.tile([P, OT, T], BF16, tag="qhi")
        Qlo = proj.tile([P, OT, T], BF16, tag="qlo")
        Khi = proj.tile([P, OT, T], BF16, tag="khi")
        Klo = proj.tile([P, OT, T], BF16, tag="klo")
        V_sb = proj.tile([P, TT, C], BF16, tag="v")
        AO_sb = proj.tile([P, OT, T], F32R, tag="ao")

        state = {"wv": None, "wo": None}
        wqk_tiles = {}

        def load_qk_chunk(ot):
            wq_ch = wqkp.tile([P, OT, P], F32R, tag="wqk")
            nc.gpsimd.dma_start(wq_ch[:], wq_d[ot].bitcast(F32R))
            wk_ch = wqkp.tile([P, OT, P], F32R, tag="wqk")
            nc.gpsimd.dma_start(wk_ch[:], wk_d[ot].bitcast(F32R))
            wqk_tiles[ot] = (wq_ch, wk_ch)

        def emit_qk_proj(ot):
            # Q^T / K^T channel tile ot -> bf16 hi/lo splits
            wq_ch, wk_ch = wqk_tiles.pop(ot)
            for w_ch, bias_sb, hi, lo in ((wq_ch, bqp_sb, Qhi, Qlo),
                                          (wk_ch, bkp_sb, Khi, Klo)):
                ps = psAT.tile([P, T], F32, tag="psAT")
                for kt in range(OT):
                    nc.tensor.matmul(
                        ps[:],
                        lhsT=w_ch[:, kt, :],
                        rhs=xR_sb[:, kt, :],
                        start=(kt == 0),
                        stop=(kt == OT - 1),
                    )
                nc.scalar.activation(hi[:, ot, :], ps[:], Identity,
                                     bias=bias_sb[:, ot:ot + 1])
                qf = qfpool.tile([P, T], F32, tag="qf")
                nc.scalar.activation(qf[:], ps[:], Identity,
                                     bias=bias_sb[:, ot:ot + 1])
                nc.gpsimd.tensor_tensor(
                    lo[:, ot, :], qf[:], hi[:, ot, :], op=sub_op)

        def emit_v_group(tt, oh):
            # V[tok, ch] tile (V bias folded into output bias host-side)
            ps = psAT.tile([P, T], F32, tag="psAT")
            for kt in range(OT):
                nc.tensor.matmul(
                    ps[:],
                    lhsT=xR_sb[:, kt, tt * P:(tt + 1) * P],
                    rhs=state["wv"][:, kt, oh * 512:(oh + 1) * 512],
                    start=(kt == 0),
                    stop=(kt == OT - 1),
                )
            nc.scalar.copy(V_sb[:, tt, oh * 512:(oh + 1) * 512], ps[:])

        # ---- tile-granular software pipeline ----
        # Tile t = 8*g + 4*hh + it. Score matmuls for tile t are emitted at
        # step t; its selection/mask lags 2 steps so every engine's in-order
        # queue only sees work whose cross-engine deps are already done.
        tile_ps = {}
        tile_E = {}
        tile_pu = {}
        pair_sums = {}

        def emit_tile_scores(t):
            g, hh, it = t // 8, (t // 4) % 2, t % 4
            h = 2 * g + hh
            prow = 64 * hh
            if t % 8 == 0:
                pair_sums[g] = headp.tile([P, 2 * TT], F32, tag="sums",
                                          name="sums_g")
            pb_sb = pbpool.tile([P, T], F32R, tag="pb")
            nc.sync.dma_start(pb_sb[:], posb_d[h, it].bitcast(F32R))
            ps = psS.tile([P, T], F32, tag="psS")
            nc.tensor.matmul(ps[:], lhsT=ident_r[:], rhs=pb_sb[:],
                             start=True, stop=False)
            terms = ((Qhi, Khi), (Qhi, Klo), (Qlo, Khi))
            for ti, (qq, kk) in enumerate(terms):
                nc.tensor.matmul(
                    ps[:],
                    lhsT=qq[prow:prow + 64, g, it * P:(it + 1) * P],
                    rhs=kk[prow:prow + 64, g, :],
                    start=False,
                    stop=(ti == len(terms) - 1),
                )
            E = epool.tile([P, T], F32, tag="E")
            nc.scalar.activation(E[:], ps[:], Exp)
            tile_ps[t] = ps
            tile_E[t] = E

        def emit_tile_select(t):
            g, hh, it = t // 8, (t // 4) % 2, t % 4
            ps = tile_ps.pop(t)
            E = tile_E.pop(t)
            sums_g = pair_sums[g]
            m_all = small.tile([P, 32], F32, tag="mall")
            sc0 = scp.tile([P, T], F32, tag="sc")
            sc1 = scp.tile([P, T], F32, tag="sc")
            sc2 = scp.tile([P, T], F32, tag="sc")
            nc.vector.max(out=m_all[:, 0:8], in_=ps[:])
            nc.vector.match_replace(
                out=sc0[:], in_to_replace=m_all[:, 0:8],
                in_values=ps[:], imm_value=-1e30)
            nc.vector.max(out=m_all[:, 8:16], in_=sc0[:])
            nc.vector.match_replace(
                out=sc1[:], in_to_replace=m_all[:, 8:16],
                in_values=sc0[:], imm_value=-1e30)
            nc.vector.max(out=m_all[:, 16:24], in_=sc1[:])
            nc.vector.match_replace(
                out=sc2[:], in_to_replace=m_all[:, 16:24],
                in_values=sc1[:], imm_value=-1e30)
            nc.vector.max(out=m_all[:, 24:32], in_=sc2[:])
            scrapM = small.tile([P, 32], F32, tag="scrapM")
            nc.scalar.activation(
                scrapM[:], m_all[:], Exp,
                accum_out=sums_g[:, 4 * hh + it:4 * hh + it + 1])
            # Sign-based mask (Pool has no comparisons): sgn is -1 on kept,
            # +1 on dropped; E - E*sgn = 2E kept / exactly 0 dropped. The
            # eps*S term keeps the 32nd element strictly on the kept side;
            # the factor 2 folds into the normalization scale.
            sgn = sgpool.tile([P, T], BF16, tag="sgn")
            nc.scalar.activation(sgn[:], ps[:],
                                 mybir.ActivationFunctionType.Sign,
                                 bias=m_all[:, 31:32],
                                 scale=-(1.0 + 5e-6))
            Em = empool.tile([P, T], F32, tag="Em")
            nc.gpsimd.tensor_tensor(Em[:], E[:], sgn[:], op=mult)
            p_u = pupool.tile([P, T], BF16, tag="P")
            nc.gpsimd.tensor_tensor(p_u[:], E[:], Em[:], op=sub_op)
            tile_pu[t] = p_u

        def emit_pair_tail(g):
            sums_g = pair_sums.pop(g)
            inv = headp.tile([P, 2 * TT], F32, tag="inv")
            nc.vector.reciprocal(inv[:], sums_g[:])
            ao_ps = psO.tile([P, T], F32, tag="psO")
            for hh in range(2):
                h = 2 * g + hh
                scl = headp.tile([P, TT], F32, tag="scl")
                nc.vector.tensor_scalar(scl[:], inv[:, hh * TT:(hh + 1) * TT],
                                        gates_sb[:, h:h + 1], 0.5,
                                        op0=mult, op1=mult)
                p_r = []
                for it in range(TT):
                    pr = prpool.tile([P, T], BF16, tag="Pr")
                    nc.scalar.activation(pr[:], tile_pu.pop(8 * g + 4 * hh + it)[:],
                                         Copy, scale=scl[:, it:it + 1])
                    p_r.append(pr)
                # software-pipelined transpose -> PT copy -> AO matmul
                pt_ps_q = []
                for jt in range(TT):
                    pt_ps = psAT.tile([P, T], BF16, tag="psAT")
                    for it in range(TT):
                        nc.tensor.transpose(
                            pt_ps[:, it * P:(it + 1) * P],
                            p_r[it][:, jt * P:(jt + 1) * P],
                            ident_b[:],
                        )
                    PT_sb = ptpool.tile([P, T], BF16, tag="PT")
                    nc.scalar.copy(PT_sb[:], pt_ps[:])
                    pt_ps_q.append(PT_sb)
                for jt in range(TT):
                    nc.tensor.matmul(
                        ao_ps[64 * hh:64 * hh + 64, :],
                        lhsT=V_sb[:, jt, h * 64:(h + 1) * 64],
                        rhs=pt_ps_q[jt][:],
                        start=(jt == 0),
                        stop=(jt == TT - 1),
                    )
            nc.scalar.copy(AO_sb[:, g, :], ao_ps[:])

        # schedules (step -> work), keeping PE fed without starving the
        # selection pipeline
        qk_sched = {}
        for ot in range(2, OT):
            s0 = 8 * (ot - 1) + 1
            qk_sched.setdefault(s0, []).append((ot, 0))
            qk_sched.setdefault(s0 + 4, []).append((ot, 1))
        chunk_sched = {8 * (ot - 2): ot for ot in range(2, OT)}
        v_sched = {3: (0, 0), 5: (1, 0), 7: (2, 0), 8: (3, 0),
                   18: (0, 1), 20: (1, 1), 22: (2, 1), 24: (3, 1)}

        load_qk_chunk(0)
        load_qk_chunk(1)
        state["wv"] = wvop.tile([P, OT, C], F32R, tag="wvo", name="wv_sb")
        nc.gpsimd.dma_start(state["wv"][:], wv_d[:].bitcast(F32R))
        emit_qk_proj(0)
        emit_qk_proj(1)

        def emit_qk_group(ot, which):
            # one of the two (q, k) projection groups for channel tile ot
            wq_ch, wk_ch = wqk_tiles[ot]
            w_ch, bias_sb, hi, lo = ((wq_ch, bqp_sb, Qhi, Qlo),
                                     (wk_ch, bkp_sb, Khi, Klo))[which]
            ps = psAT.tile([P, T], F32, tag="psAT")
            for kt in range(OT):
                nc.tensor.matmul(
                    ps[:],
                    lhsT=w_ch[:, kt, :],
                    rhs=xR_sb[:, kt, :],
                    start=(kt == 0),
                    stop=(kt == OT - 1),
                )
            nc.scalar.activation(hi[:, ot, :], ps[:], Identity,
                                 bias=bias_sb[:, ot:ot + 1])
            qf = qfpool.tile([P, T], F32, tag="qf")
            nc.scalar.activation(qf[:], ps[:], Identity,
                                 bias=bias_sb[:, ot:ot + 1])
            nc.gpsimd.tensor_tensor(
                lo[:, ot, :], qf[:], hi[:, ot, :], op=sub_op)
            if which == 1:
                wqk_tiles.pop(ot)

        for s in range(66):
            if s in chunk_sched:
                load_qk_chunk(chunk_sched[s])
            for ot, which in qk_sched.get(s, ()):
                emit_qk_group(ot, which)
            if s in v_sched:
                emit_v_group(*v_sched[s])
            if s < 64:
                emit_tile_scores(s)
            t = s - 2
            if 0 <= t < 64:
                emit_tile_select(t)
                if t % 8 == 7:
                    emit_pair_tail(t // 8)
            if s == 30:
                state["wo"] = wvop.tile([P, OT, C], F32R, tag="wvo",
                                        name="wo_sb")
                nc.gpsimd.dma_start(state["wo"][:], wo_d[:].bitcast(F32R))

        # ---- output projection (f32r) ----
        for tt in range(TT):
            for oh in range(2):
                ps = psAT.tile([P, T], F32, tag="psAT")
                nc.tensor.matmul(ps[:], lhsT=ident_r[:],
                                 rhs=bob_r[:, oh * 512:(oh + 1) * 512],
                                 start=True, stop=False)
                for ct in range(OT):
                    nc.tensor.matmul(
                        ps[:],
                        lhsT=AO_sb[:, ct, tt * P:(tt + 1) * P],
                        rhs=state["wo"][:, ct, oh * 512:(oh + 1) * 512],
                        start=False,
                        stop=(ct == OT - 1),
                    )
                o_sb = outp.tile([P, T], F32, tag="o")
                nc.scalar.copy(o_sb[:], ps[:])
                nc.sync.dma_start(out_d[tt * P:(tt + 1) * P,
                                        oh * 512:(oh + 1) * 512], o_sb[:])

    nc.compile()
    if not nc.is_finalized():
        nc.finalize()
    return nc


def prep_inputs(x, Wq, bq, Wk, bk, Wv, bv, Wo, bo, head_gates, rel_bias):
    """Host-side reshapes/transposes into the layouts the device program wants."""
    x = np.asarray(x, np.float32)
    scale = np.float32(1.0 / np.sqrt(D))

    def to_kpart(w):
        # [C_in, C_out] -> [P, OT, C_out] with c_in = kt*P + p
        return np.ascontiguousarray(
            np.asarray(w, np.float32).reshape(OT, P, C).transpose(1, 0, 2))

    def to_kpart_chunked(w):
        # [C_in, C_out] -> [OT_out, P, OT_kt, P]
        return np.ascontiguousarray(
            np.asarray(w, np.float32).reshape(OT, P, OT, P)
            .transpose(2, 1, 0, 3))

    wq_r = to_kpart_chunked(np.asarray(Wq, np.float32).T * scale)
    wk_r = to_kpart_chunked(np.asarray(Wk, np.float32).T)
    wv_r = to_kpart(np.asarray(Wv, np.float32).T)
    wo_r = to_kpart(np.asarray(Wo, np.float32).T)

    bqp = np.ascontiguousarray((np.asarray(bq, np.float32) * scale).reshape(OT, P).T)
    bkp = np.ascontiguousarray(np.asarray(bk, np.float32).reshape(OT, P).T)
    # V bias folded into the output bias: the normalized gated weights of each
    # head sum to exactly gate_h, so attn_out carries a constant gate_h * bv_h
    # per head, which maps through Wo^T into a constant output bias.
    g64 = np.repeat(np.asarray(head_gates, np.float64), D)
    bo_eff = (np.asarray(bo, np.float64)
              + (g64 * np.asarray(bv, np.float64)) @ np.asarray(Wo, np.float64).T)
    bob = np.ascontiguousarray(
        np.tile(bo_eff.astype(np.float32)[None, :], (P, 1)))
    gates = np.ascontiguousarray(
        np.tile(np.asarray(head_gates, np.float32)[None, :], (P, 1)))

    idx = np.arange(T)
    rel = idx[None, :] - idx[:, None] + (MAX_POS - 1)          # [T, T]
    pb = np.asarray(rel_bias, np.float32)[rel] + np.float32(SHIFT)  # [T, T, H]
    posb = np.ascontiguousarray(
        pb.transpose(2, 0, 1).reshape(H, TT, P, T))            # [H, TT, P, T]

    shared = dict(wq=wq_r, wk=wk_r, wv=wv_r, wo=wo_r, bqp=bqp, bkp=bkp,
                  bob=bob, gates=gates, posb=posb)

    in_maps = []
    for b in range(B):
        xT = np.ascontiguousarray(
            x[b].T.reshape(OT, P, T).transpose(1, 0, 2))       # [P, OT, T]
        in_maps.append(dict(xT=xT, **shared))
    return in_maps


_NC_CACHE = {}


def get_program():
    if "nc" not in _NC_CACHE:
        _NC_CACHE["nc"] = build_program()
    return _NC_CACHE["nc"]


def kernel(x, Wq, bq, Wk, bk, Wv, bv, Wo, bo, head_gates, rel_bias):
    nc = get_program()
    in_maps = prep_inputs(x, Wq, bq, Wk, bk, Wv, bv, Wo, bo, head_gates, rel_bias)
    res = run_bass_kernel_spmd(nc, in_maps, list(range(N_CORES)))
    return np.stack([res.results[b]["out"] for b in range(B)], axis=0)
